# revision 101
# baseline (speedup 1.0000x reference)
"""Trainium2 Bass kernel: 3x3 VALID conv (NHWC) with weight thresholding + bias.

Full-input contract: kernel(x, weight, bias) -> out
  x:      (32, 56, 56, 256) fp32 NHWC
  weight: (256, 256, 3, 3)  fp32 OIHW, |w| < 0.01 -> 0
  bias:   (256,)            fp32
  out:    (32, 54, 54, 256) fp32 NHWC

Sharding: data-parallel over batch, 4 images per core on 8 cores.

Algorithm: 1D Winograd F(6,3) along H (vertical taps), direct 3-tap
accumulation along W in PSUM. 56 input rows = 9 tiles x 6 + 2 exactly;
54 output rows = 9 x 6 exactly (no edge cases).

Per image and ci-chunk the input transform builds V[xi, vt, w] =
sum_d BT[xi,d] x[6vt+d, w] (xi in 0..7). The conv becomes, per output
row-group: M[xi][co, (vt,w)] = sum_{kw,ci} U[xi,kw]^T V[ci, xi, vt, w+kw]
(6 accumulating matmuls of N=486 per xi per co-chunk), and the output
transform y[6vt+i, w] = sum_xi AT[i,xi] M[xi][vt, w].

PE work drops 2.25x vs direct conv (4 streamed columns per output elem
instead of 9). All matmul operands are fp16 (1 col/cycle, like fp32r but
enabling the DVE 2-byte 2x mode for the transform adds); PSUM accumulates
fp32. Numerics simulated at 0.22% rel err vs the fp32 reference (gate 2e-2).

Engine split (scalar_tensor_tensor is DVE-only on real HW — the neuronxcc
ISA check rejects it on Pool, so all scale+add ops live on DVE):
  PE:   48 matmuls / image (8 xi x 3 kw x 2 ci-chunks, per co-chunk)
  Act:  scaled copies (5*d rows, 17/4- and 21/4- and 2x-scaled
        intermediates) + 8 PSUM->SBUF fp16 drains per (img, co-chunk);
        bias fused into the D1 drain (AT column 1 is all-ones so bias
        lands once in every output row)
  DVE:  fp16 tensor_tensor adds (2-byte 2x mode) for chained transform
        sums, p/m pair reductions, y0/y5 chains, and ALL y1..y5
        scalar_tensor_tensor rows
  Pool: x-direct tensor_tensor sums of the input transform (its only
        legal elementwise op; runs at 0.42 efficiency)

The issue order is software-pipelined so each engine's in-order queue is
near wait-free: image img+1's input transform is issued interleaved with
image img's matmul/drain/output-transform stages (see the pipeline loop).
A 4x-scaled packed copy of x (row positions {1,2,5,6} mod 6) is shipped
from the host so the V3..V6 chains need no 4x multiplies on-device.

V rows 3..6 are computed scaled (lambda = [1,1,1,4,4,2,2,1]); the scales are
divided out of the host-side weight transform U = G w / lambda.

Host marshaling: x -> fp16 channel-major [128, img, ci_chunk, pix]; weights
transformed+packed to fp16 [128, 96*128]; output returned fp16
[256, img, i, vt, w] and unpermuted/cast on host.
"""

import numpy as np
from contextlib import ExitStack

import concourse.bass as bass
import concourse.bacc as bacc
import concourse.tile as tile
import concourse.mybir as mybir
from concourse.bass_utils import run_bass_kernel_spmd

N_CORES = 8
IMGS = 4
H, W, C = 56, 56, 256
OH, OW, CO = 54, 54, 256
NPIX = H * W          # 3136
P = 128
XI = 8                # winograd tile points
VT = 9                # vertical tiles per image
BLK = VT * OW         # 486 columns per M tile
SPARSE_TH = 0.01

# F(6,3) transform matrices (points 0, +-1, +-2, +-1/2, inf)
G_MAT = np.array([
    [1, 0, 0],
    [-2 / 9, -2 / 9, -2 / 9],
    [-2 / 9, 2 / 9, -2 / 9],
    [1 / 90, 1 / 45, 2 / 45],
    [1 / 90, -1 / 45, 2 / 45],
    [32 / 45, 16 / 45, 8 / 45],
    [32 / 45, -16 / 45, 8 / 45],
    [0, 0, 1]], dtype=np.float64)
LAMBDA = np.array([1, 1, 1, 4, 4, 2, 2, 1], dtype=np.float64)

TRACE = False
LAST = None  # BassKernelResults of the most recent run (for test harness)

_NC_CACHE = None


def _build_module():
    f32 = mybir.dt.float32
    f16 = mybir.dt.float16
    Alu = mybir.AluOpType
    ACT_IDENT = mybir.ActivationFunctionType.Identity

    nc = bacc.Bacc(
        "TRN2",
        target_bir_lowering=False,
        debug=False,
        enable_asserts=False,
        num_devices=N_CORES,
    )
    NX4 = 2 * 4 * VT * W  # x4 packed: only row positions {1,2,5,6} are read
    xt = nc.dram_tensor("xt", [P, IMGS * 2 * NPIX], f16, kind="ExternalInput").ap()
    xt4 = nc.dram_tensor("xt4", [P, IMGS * NX4], f16, kind="ExternalInput").ap()
    wp = nc.dram_tensor("wp", [P, 96 * P], f16, kind="ExternalInput").ap()
    b2 = nc.dram_tensor("b2", [P, 2], f32, kind="ExternalInput").ap()
    yt = nc.dram_tensor("yt", [CO, IMGS * 6 * BLK], f16, kind="ExternalOutput").ap()

    with tile.TileContext(nc) as tc, ExitStack() as ctx:
        wpool = ctx.enter_context(tc.tile_pool(name="w", bufs=1))
        bpool = ctx.enter_context(tc.tile_pool(name="b", bufs=1))
        xpool = ctx.enter_context(tc.tile_pool(name="x", bufs=3))
        x4pool = ctx.enter_context(tc.tile_pool(name="x4", bufs=2))
        vpool = ctx.enter_context(tc.tile_pool(name="v", bufs=2))
        ipool = ctx.enter_context(tc.tile_pool(name="i", bufs=1))
        dpool = ctx.enter_context(tc.tile_pool(name="d", bufs=3))
        opool = ctx.enter_context(tc.tile_pool(name="o", bufs=2))
        ypool = ctx.enter_context(tc.tile_pool(name="y", bufs=2))
        pspool = ctx.enter_context(tc.tile_pool(name="ps", bufs=1, space="PSUM"))

        w_sb = wpool.tile([P, 96 * P], f16)
        b_sb = bpool.tile([P, 2], f32)

        NI = 2 * VT * W  # 1008 elements per intermediate (both ci chunks)

        def itile(name, img):
            t = ipool.tile([P, NI], f16, tag=name, name=f"{name}_{img}")
            return t[:].rearrange("p (c v w) -> p c v w", c=2, v=VT, w=W)

        # matmul consumption order: V1 first (its transform deps are cheapest);
        # D7 before D0 so the long y5 chain starts earlier
        XI_ORDER = (1, 2, 3, 4, 5, 6, 7, 0)
        # weight DMA split per xi so the first matmuls aren't gated on the
        # full 3.1 MB weight transfer; chunks interleaved with x prefetches
        WCHUNK = 12 * P  # one xi: 3 kw x 2 ci x 2 co tiles

        def load_w_xi(xi, eng=None):
            (eng or nc.sync).dma_start(
                w_sb[:, xi * WCHUNK:(xi + 1) * WCHUNK],
                wp[:, xi * WCHUNK:(xi + 1) * WCHUNK])

        # per-image views, filled by issue_input_dma
        xviews = {}
        vviews = {}

        x4views = {}

        def issue_xc_dma(img, mid=lambda: None):
            xc = xpool.tile([P, 2 * NPIX], f16, tag="x", name=f"x_{img}")
            base = img * 2 * NPIX
            nc.sync.dma_start(xc[:, :NPIX], xt[:, base:base + NPIX])
            mid()  # urgent small DMAs jump ahead of the bulk transfers
            nc.sync.dma_start(xc[:, NPIX:], xt[:, base + NPIX:base + 2 * NPIX])
            xr = xc[:].rearrange("p (c h w) -> p c h w", c=2, h=H, w=W)
            SL = [slice(di, di + 6 * (VT - 1) + 1, 6) for di in range(8)]
            xviews[img] = [xr[:, :, s, :] for s in SL]

        def issue_x4_dma(img, eng=None):
            xc4 = x4pool.tile([P, NX4], f16, tag="x4", name=f"x4_{img}")
            base = img * NX4
            (eng or nc.sync).dma_start(xc4[:], xt4[:, base:base + NX4])
            xr4 = xc4[:].rearrange("p (c k v w) -> p c k v w",
                                   c=2, k=4, v=VT, w=W)
            # packed k order: 0->d1, 1->d2, 2->d5, 3->d6
            x4views[img] = {1: xr4[:, :, 0], 2: xr4[:, :, 1],
                            5: xr4[:, :, 2], 6: xr4[:, :, 3]}

        _itc = {}

        def _it(img, name, cs):
            if (img, name) not in _itc:
                _itc[(img, name)] = itile(name, img)
            return _itc[(img, name)][:, cs]

        def issue_prescales(img, chunk=None):
            cs = slice(None) if chunk is None else slice(chunk, chunk + 1)
            d = xviews[img]
            for name, di, s in (("c5d4", 4, 5.0), ("c5d3", 3, 5.0)):
                nc.scalar.mul(_it(img, name, cs), d[di][:, cs], s)

        def issue_input_xform(img, chunk=None,
                              parts=("p1", "cs", "p2a", "c2", "v12", "v36",
                                     "p3")):
            # chunk=None: both ci chunks in one set of ops (steady state).
            # chunk=0/1: per-chunk ops (image 0 priming — the first ops can
            # start as soon as the first half of x lands, and per-chunk V
            # rows unblock the ci=0 matmuls early).
            # parts are engine-homogeneous groups so the steady-state loop
            # can place each at the right point of its engine's queue:
            #   p1  (DVE): t/u/s sums          cs  (Act): 17/4-scaled copies
            #   p2a (DVE): V3..V6 chain sums   c2  (Act): 2x-scaled copies
            #   v12 (DVE): V1/V2 writes        v36 (DVE): V3..V6 writes
            #   p3  (DVE+Pool): V0/V7
            cs_ = slice(None) if chunk is None else slice(chunk, chunk + 1)
            d = [v[:, cs_] for v in xviews[img][:8]]
            d4x = x4views[img]
            c4d5, c4d2, c4d1, c4d6 = (
                d4x[i][:, cs_] for i in (5, 2, 1, 6))
            if chunk in (None, 0) and "p1" in parts:
                vtile = vpool.tile([P, 2 * XI * VT * W], f16, tag="V",
                                   name=f"V_{img}")
                vviews[img] = vtile[:].rearrange(
                    "p (c x v w) -> p c x v w", c=2, x=XI, v=VT, w=W)
            vr = vviews[img][:, cs_]

            def it(name):
                return _it(img, name, cs_)

            if "p1" in parts:
                # V1/V2 feed the FIRST matmuls of the next image — keep their
                # whole dep chain on DVE (a serial 2.1us Pool op here becomes
                # a PE stall at the image boundary)
                t1 = it("t1"); t2 = it("t2"); u = it("u"); s1 = it("s1")
                nc.vector.tensor_tensor(t1, d[2], d[6], op=Alu.add)
                nc.vector.tensor_tensor(t2, d[1], d[5], op=Alu.add)
                nc.vector.tensor_tensor(u, t1, t2, op=Alu.add)
                nc.vector.tensor_tensor(s1, d[3], d[4], op=Alu.add)
                um = it("um"); s2 = it("s2")
                nc.vector.tensor_tensor(um, t1, t2, op=Alu.subtract)
                nc.vector.tensor_tensor(s2, d[3], d[4], op=Alu.subtract)
            if "cs" in parts:
                # 17/4-scaled copies feeding V1/V2 (Act has slack; each
                # converts a DVE stt into a cheaper fp16 2x tensor_tensor)
                nc.scalar.mul(it("cs1"), it("s1"), -4.25)
                nc.scalar.mul(it("cs2"), it("s2"), 4.25)
            if "v12pool" in parts:
                # prologue only: V1/V2 directly on Pool (no Act round-trip,
                # Pool is idle at startup)
                nc.vector.scalar_tensor_tensor(
                    vr[:, :, 1], it("s1"), -4.25, it("u"),
                    op0=Alu.mult, op1=Alu.add)
                nc.vector.scalar_tensor_tensor(
                    vr[:, :, 2], it("s2"), 4.25, it("um"),
                    op0=Alu.mult, op1=Alu.add)
            if "p2a" in parts:
                c5d4 = it("c5d4")
                c5d3 = it("c5d3")
                #   rr = (d1 - 5d3) + 4d5,  4a = (d2 - 5d4) + 4d6
                #   g2 = (4d1 + d5) - 5d3,  c = (4d2 + d6) - 5d4
                p = it("p"); q = it("q"); rr = it("rr"); aP = it("aP")
                nc.gpsimd.tensor_tensor(p, d[2], c5d4, op=Alu.subtract)
                nc.gpsimd.tensor_tensor(q, d[1], c5d3, op=Alu.subtract)
                nc.vector.tensor_tensor(rr, q, c4d5, op=Alu.add)
                nc.vector.tensor_tensor(aP, p, c4d6, op=Alu.add)
                tmp = it("tmp"); cc = it("cc"); tmp2 = it("tmp2")
                g2 = it("g2")
                nc.gpsimd.tensor_tensor(tmp, c4d2, d[6], op=Alu.add)
                nc.gpsimd.tensor_tensor(cc, tmp, c5d4, op=Alu.subtract)
                nc.gpsimd.tensor_tensor(tmp2, c4d1, d[5], op=Alu.add)
                nc.gpsimd.tensor_tensor(g2, tmp2, c5d3, op=Alu.subtract)
            if "c2" in parts:
                nc.scalar.mul(it("c2rr"), it("rr"), 2.0)
                nc.scalar.mul(it("c2c"), it("cc"), 2.0)
            if "v12" in parts:
                # V1 = u - 17/4 s1;  V2 = um + 17/4 s2
                nc.vector.tensor_tensor(vr[:, :, 1], it("u"), it("cs1"),
                                        op=Alu.add)
                nc.vector.tensor_tensor(vr[:, :, 2], it("um"), it("cs2"),
                                        op=Alu.add)
            if "v36" in parts:
                # V3' = 4a + 2rr, V4' = 4a - 2rr (lambda=4)
                # V5' = g2 + 2c,  V6' = 2c - g2  (lambda=2)
                aP = it("aP"); c2rr = it("c2rr")
                g2 = it("g2"); c2c = it("c2c")
                nc.vector.tensor_tensor(vr[:, :, 3], aP, c2rr, op=Alu.add)
                nc.vector.tensor_tensor(vr[:, :, 4], aP, c2rr, op=Alu.subtract)
                nc.vector.tensor_tensor(vr[:, :, 5], g2, c2c, op=Alu.add)
                nc.vector.tensor_tensor(vr[:, :, 6], c2c, g2, op=Alu.subtract)
            if "p3" in parts:
                # V0 = (d0-d6) + 21/4 (d4-d2);  V7 = (d7-d1) + 21/4 (d3-d5)
                # V1 = u - 17/4 s1 (Pool, balancing the engine loads)
                A0 = it("A0"); B0 = it("B0"); A7 = it("A7"); B7 = it("B7")
                nc.gpsimd.tensor_tensor(A0, d[0], d[6], op=Alu.subtract)
                nc.gpsimd.tensor_tensor(B0, d[4], d[2], op=Alu.subtract)
                cb0 = itile("cb", f"{img}a")[:, cs_]
                nc.scalar.mul(cb0, B0, 5.25)
                nc.vector.tensor_tensor(vr[:, :, 0], A0, cb0, op=Alu.add)
                nc.gpsimd.tensor_tensor(A7, d[7], d[1], op=Alu.subtract)
                nc.gpsimd.tensor_tensor(B7, d[3], d[5], op=Alu.subtract)
                cb7 = itile("cb", f"{img}b")[:, cs_]
                nc.scalar.mul(cb7, B7, 5.25)
                nc.vector.tensor_tensor(vr[:, :, 7], A7, cb7, op=Alu.add)

        dtiles = {}
        ydefer = {}

        # drain layout: odd xis (1,3,5) -> DA slices, even (2,4,6) -> DB
        # slices, so the p/m pair reductions are 2 wide DVE ops instead of 6
        DSLOT = {1: 0, 3: 1, 5: 2, 2: 0, 4: 1, 6: 2}

        def issue_unit_mms(img, o, split_ci=False, inject=None,
                           xi_order=XI_ORDER):
            vr = vviews[img]
            DA = dpool.tile([P, 3 * BLK], f16, tag="DA", name=f"DA_{img}_{o}")
            DB = dpool.tile([P, 3 * BLK], f16, tag="DB", name=f"DB_{img}_{o}")
            D = {'A': DA, 'B': DB}
            pstiles = {}
            if split_ci:
                # startup: all ci=0 matmuls first so the PE isn't blocked
                # mid-accumulation waiting for the second x chunk's V rows
                for xi in xi_order:
                    ps = pstiles[xi] = pspool.tile(
                        [P, BLK], f32, tag=f"ps{xi}", name=f"ps_{img}_{o}_{xi}")
                    for kw in range(3):
                        t = ((xi * 3 + kw) * 2 + 0) * 2 + o
                        nc.tensor.matmul(
                            ps[:], w_sb[:, t * P:(t + 1) * P],
                            vr[:, 0, xi, :, kw:kw + OW],
                            start=(kw == 0), stop=False)
            for xi in xi_order:
                if split_ci:
                    ps = pstiles[xi]
                    for kw in range(3):
                        t = ((xi * 3 + kw) * 2 + 1) * 2 + o
                        nc.tensor.matmul(
                            ps[:], w_sb[:, t * P:(t + 1) * P],
                            vr[:, 1, xi, :, kw:kw + OW],
                            start=False, stop=(kw == 2))
                else:
                    ps = pspool.tile([P, BLK], f32, tag=f"ps{xi}",
                                     name=f"ps_{img}_{o}_{xi}")
                    for mm, (ci, kw) in enumerate(
                            (ci, kw) for ci in range(2) for kw in range(3)):
                        t = ((xi * 3 + kw) * 2 + ci) * 2 + o
                        rhs = vr[:, ci, xi, :, kw:kw + OW]
                        nc.tensor.matmul(
                            ps[:], w_sb[:, t * P:(t + 1) * P], rhs,
                            start=(mm == 0), stop=(mm == 5))
                if xi in (0, 7):
                    Dx = dpool.tile([P, BLK], f16, tag=f"D{xi}",
                                    name=f"D_{img}_{o}_{xi}")
                    nc.scalar.copy(Dx[:], ps[:])
                    D[xi] = Dx
                    continue
                cat = DA if xi % 2 == 1 else DB
                sl = cat[:, DSLOT[xi] * BLK:(DSLOT[xi] + 1) * BLK]
                if xi == 1:
                    nc.scalar.activation(
                        sl, ps[:], ACT_IDENT, bias=b_sb[:, o:o + 1])
                else:
                    nc.scalar.copy(sl, ps[:])
                if inject and xi in inject:
                    inject[xi]()
            dtiles[(img, o)] = D

        def issue_out_xform(img, o, split_tail=False):
            # split_tail: pipeline tail (no next-image input transform left),
            # DVE is the critical path while Pool and Act idle — route the
            # y rows through Act scaled-copies + Pool adds instead of DVE
            # scalar_tensor_tensor, and the pair/row reductions through Pool.
            D = dtiles.pop((img, o))

            def otile(name, dt=f16):
                return opool.tile([P, BLK], dt, tag=name,
                                  name=f"{name}_{img}_{o}")

            ttv = nc.vector
            pall = opool.tile([P, 3 * BLK], f16, tag="pall",
                              name=f"pall_{img}_{o}")
            mall = opool.tile([P, 3 * BLK], f16, tag="mall",
                              name=f"mall_{img}_{o}")
            ttv.tensor_tensor(pall[:], D['A'][:], D['B'][:], op=Alu.add)
            ttv.tensor_tensor(mall[:], D['A'][:], D['B'][:], op=Alu.subtract)
            p1, p2, p3 = (pall[:, i * BLK:(i + 1) * BLK] for i in range(3))
            m1, m2, m3 = (mall[:, i * BLK:(i + 1) * BLK] for i in range(3))

            ycat = ypool.tile([P, 6 * BLK], f16, tag="y", name=f"y_{img}_{o}")
            yv = [ycat[:, i * BLK:(i + 1) * BLK] for i in range(6)]
            # y0 = D0 + p1 + p2 + p3
            t01 = otile("t01"); t02 = otile("t01")  # same tag: bufs=2 rotate
            ttv.tensor_tensor(t01[:], p1, p2, op=Alu.add)
            ttv.tensor_tensor(t02[:], t01[:], p3, op=Alu.add)
            ttv.tensor_tensor(yv[0], t02[:], D[0][:], op=Alu.add)
            # y1 = m1 + 2 m2 + m3/2 ... y4; y5 adds D7
            for i, (pm1, pm23, ca, cb) in enumerate((
                    (m1, (m2, m3), 2.0, 0.5),
                    (p1, (p2, p3), 4.0, 0.25),
                    (m1, (m2, m3), 8.0, 0.125),
                    (p1, (p2, p3), 16.0, 0.0625),
                    (m1, (m2, m3), 32.0, 1.0 / 32.0)), start=1):
                if split_tail:
                    # Act: scaled copies; Pool: adds (DVE-free row)
                    sa = otile("tav"); sb = otile("tav")
                    nc.scalar.mul(sa[:], pm23[0], ca)
                    nc.scalar.mul(sb[:], pm23[1], cb)
                    tp = otile("t01")
                    nc.gpsimd.tensor_tensor(tp[:], pm1, sa[:], op=Alu.add)
                    if i == 5:
                        tp2 = otile("t01")
                        nc.gpsimd.tensor_tensor(tp2[:], tp[:], sb[:], op=Alu.add)
                        nc.gpsimd.tensor_tensor(yv[i], tp2[:], D[7][:], op=Alu.add)
                    else:
                        nc.gpsimd.tensor_tensor(yv[i], tp[:], sb[:], op=Alu.add)
                    continue
                # scalar_tensor_tensor is DVE-only on real HW
                ta = otile("tav")
                nc.vector.scalar_tensor_tensor(
                    ta[:], pm23[0], ca, pm1, op0=Alu.mult, op1=Alu.add)
                if i == 5:
                    tb = otile("tav")
                    nc.vector.scalar_tensor_tensor(
                        tb[:], pm23[1], cb, ta[:], op0=Alu.mult, op1=Alu.add)
                    nc.vector.tensor_tensor(yv[i], tb[:], D[7][:], op=Alu.add)
                else:
                    nc.vector.scalar_tensor_tensor(
                        yv[i], pm23[1], cb, ta[:], op0=Alu.mult, op1=Alu.add)

            ydefer[(img, o)] = ycat

        def issue_ydma(img, o, piece):
            # y DMA issues go on the Act queue one pipeline iteration after
            # the data was produced: a dma_start WAITS for its writers while
            # holding the queue, so issuing it promptly would block the next
            # unit's PSUM drains behind it and stall the PE
            ycat = ydefer[(img, o)]
            col0 = img * 6 * BLK + piece * 3 * BLK
            nc.scalar.dma_start(
                yt[o * P:(o + 1) * P, col0:col0 + 3 * BLK],
                ycat[:, piece * 3 * BLK:(piece + 1) * 3 * BLK])

        # ---- software pipeline ----
        # Per-engine queues execute in issue order, so the issue points below
        # are chosen to give each engine a wait-free queue in steady state:
        #   PE:   mms(img,0) | mms(img,1)
        #   Act:  pre(img+1) | ydma(img-1,0) | dr(img,0) 1,2 | cs(img+1) |
        #         dr 3 | c2(img+1) | dr 4..0 | ydma pieces | dr(img,1)
        #   DVE:  S4(img-1,1) | p1(img+1) | p2a(img+1) | v12 | v36 |
        #         S4(img,0) | p3(img+1)
        #   Pool: ystt(img-1,1) | ystt(img,0) | V0/V7(img+1)
        #   SP:   x4(img+1) | xc(img+2)
        # ---- prologue: startup is DMA-bandwidth-bound, so spread the
        # transfers over three DGE queues that run concurrently:
        #   SP:  x image stream      Act: weight chunks + bias
        #   DVE: packed 4x copies
        # (all issued before any consumer — tile deps follow issue order;
        # a later-issued DMA write is a race, caught by CoreSim)
        PRO_PARTS = ("p1", "v12pool", "p2a", "c2", "v36", "p3")
        for xi in (1, 2, 3, 4):
            load_w_xi(xi, eng=nc.scalar)
        nc.scalar.dma_start(b_sb[:], b2)
        issue_xc_dma(0)
        issue_x4_dma(0, eng=nc.gpsimd)
        issue_xc_dma(1)
        issue_x4_dma(1, eng=nc.gpsimd)
        issue_prescales(0, chunk=0)
        issue_input_xform(0, chunk=0, parts=PRO_PARTS)
        for xi in (5, 6, 7, 0):
            load_w_xi(xi, eng=nc.scalar)
        issue_prescales(0, chunk=1)
        issue_input_xform(0, chunk=1, parts=PRO_PARTS)
        for img in range(IMGS):
            nxt = img + 1 < IMGS
            if nxt:
                issue_prescales(img + 1)
                if img + 1 >= 2:
                    issue_x4_dma(img + 1)
            if img + 2 < IMGS:
                issue_xc_dma(img + 2)
            if img >= 1:
                issue_ydma(img - 1, 0, 0)
                # previous image's second-half output transform: its drains
                # only finish after mms(img-1,1), i.e. at the start of this
                # iteration — issuing it earlier would stall the DVE queue
                issue_out_xform(img - 1, 1, split_tail=False)
            if nxt:
                issue_input_xform(img + 1, parts=("p1",))
                issue_input_xform(img + 1, parts=("p2a",))
            inj = {}
            if nxt:
                inj[2] = lambda i=img: issue_input_xform(i + 1, parts=("cs",))
                inj[3] = lambda i=img: issue_input_xform(i + 1, parts=("c2",))
            if img >= 1:
                inj[4] = lambda i=img: issue_ydma(i - 1, 0, 1)
                inj[5] = lambda i=img: issue_ydma(i - 1, 1, 0)
            issue_unit_mms(img, 0, split_ci=(img == 0), inject=inj)
            if nxt:
                issue_input_xform(img + 1, parts=("v12",))
                issue_input_xform(img + 1, parts=("v36",))
            inj2 = {}
            if img >= 1:
                inj2[2] = lambda i=img: issue_ydma(i - 1, 1, 1)
            issue_unit_mms(img, 1, inject=inj2)
            issue_out_xform(img, 0)
            if nxt:
                issue_input_xform(img + 1, parts=("p3",))
        issue_ydma(IMGS - 1, 0, 0)
        issue_ydma(IMGS - 1, 0, 1)
        issue_out_xform(IMGS - 1, 1)
        issue_ydma(IMGS - 1, 1, 0)
        issue_ydma(IMGS - 1, 1, 1)
    nc.compile()
    return nc


def _host_weights(weight):
    w = np.where(np.abs(weight) < SPARSE_TH, 0.0, weight).astype(np.float64)
    U = np.einsum('xh,ochw->xwco', G_MAT, w)          # [xi, kw, ci, co]
    U = U / LAMBDA[:, None, None, None]
    U16 = U.astype(np.float16)
    Ur = U16.reshape(XI, 3, 2, P, 2, P)               # [xi,kw,cich,ciin,coch,coin]
    wp = np.ascontiguousarray(
        Ur.transpose(3, 0, 1, 2, 4, 5).reshape(P, 96 * P))
    return wp


def kernel(x, weight, bias):
    global _NC_CACHE, LAST
    x = np.ascontiguousarray(np.asarray(x, dtype=np.float32))
    weight = np.asarray(weight, dtype=np.float32)
    bias = np.asarray(bias, dtype=np.float32)

    wp = _host_weights(weight)
    b2 = np.ascontiguousarray(bias.reshape(2, P).T.astype(np.float32))

    in_maps = []
    for i in range(N_CORES):
        xc = x[i * IMGS:(i + 1) * IMGS]               # [4,56,56,256]
        xt_i = np.ascontiguousarray(
            xc.reshape(IMGS, NPIX, 2, P)
            .transpose(3, 0, 2, 1)
            .reshape(P, IMGS * 2 * NPIX).astype(np.float16))
        # packed 4x copy: only the row positions {1,2,5,6} mod 6 that the
        # V3..V6 transform chains read
        x4r = (xt_i * np.float16(4.0)).reshape(P, IMGS, 2, H, W)
        xt4_i = np.ascontiguousarray(
            np.stack([x4r[:, :, :, k:k + 6 * (VT - 1) + 1:6, :]
                      for k in (1, 2, 5, 6)], axis=3)
            .reshape(P, IMGS * 2 * 4 * VT * W))
        in_maps.append({"xt": xt_i, "xt4": xt4_i, "wp": wp, "b2": b2})

    if _NC_CACHE is None:
        _NC_CACHE = _build_module()
    nc = _NC_CACHE

    LAST = run_bass_kernel_spmd(
        nc, in_maps, core_ids=list(range(N_CORES)), trace=TRACE
    )

    out = np.empty((32, OH, OW, CO), np.float32)
    for i in range(N_CORES):
        ytc = LAST.results[i]["yt"]                   # [256, 4*6*9*54] fp16
        y = ytc.reshape(2, P, IMGS, 6, VT, OW).astype(np.float32)
        # out[img, 6v+i, w, ch*128+coin] = y[ch, coin, img, i, v, w]
        out[i * IMGS:(i + 1) * IMGS] = (
            y.transpose(2, 4, 3, 5, 0, 1).reshape(IMGS, OH, OW, CO))
    return out
